# revision 32
# baseline (speedup 1.0000x reference)
"""Trainium2 Bass kernel for nn_GNN_82781199663565 (gnn_message_passing).

Computation (see reference):
  du = relu(BN(einsum(h_att[1]*xp, Wu)))   # [B, 40, H, W]
  dl = relu(BN(einsum(h_att[2]*xp, Wl)))   # [B, 20, H, W]
  p_new[0]   = 0.5*(h_nodes[0] + p_nodes[0])
  p_new[1:5] = 0.5*(p_nodes[1:5] + du4)    # du reshaped to [4, B, 10, H, W]
  p_new[5:7] = 0.5*(p_nodes[5:7] + dl2)
(f_nodes, h_att[0], h_nodes[1:] are unused.)

Strategy: data-parallel over H (32 rows per core, 8 cores). Per core:
 - All bulk HBM traffic in bf16 (harness gate is rel_err < 2e-2): xp,
   attention, p_nodes, outputs. Stats/affine math stays f32.
 - One fused matmul z = Wcat.T @ xp per 512-col window; the two batch
   images run CONCURRENTLY on separate PE column halves via
   tile_position=(0,0)/(0,64) -> halves PE streaming time.
 - Attention applied after the conv (channel-independent) via one fused
   vector op that also accumulates the per-partition sum for BN stats.
 - Sync-BN via an XOR-pattern SBUF->SBUF remote-DMA exchange: each core
   broadcasts its [128,2] partial sums to peer (own_tpb ^ j) for
   j=1..7, receives 7 peer blocks, reduces locally. Replaces the
   gpsimd collective_compute AllReduce (~43us bubble -> a few us).
 - p_new = relu_affine(y) + 0.5*p_nodes in one fused vector op.
All host-side work is layout only (slice/transpose/pad/concat/cast).
"""
import sys
sys.path.insert(0, '/opt/trn_rl_repo')

import numpy as np
import ml_dtypes

N_CORES = 8
B, C, HID, H, W = 2, 256, 10, 256, 256
EPS = 1e-5
HS = H // N_CORES            # 32 H-rows per core
SPB = HS * W                 # spatial elems per batch image per core: 8192
M = 60                       # real output channels (40 u + 20 l)
MP = 64                      # padded to 64 -> groups tile partitions exactly
PP = 128
NQ = 1024                    # phase-3 columns per tile
NB = 512                     # matmul free-dim block (one PSUM bank, fp32)
NTOT = float(B * H * W)      # BN stat count: 131072
XN = 4096                    # xp super-tile columns (1 MiB bf16 DMAs)
QS = SPB // XN               # 2 super-iterations
NW = XN // NB                # 8 windows per super-iteration

# packed f32 constants column offsets: foldW, bcW, gamma, beta
C_FOLD = 0
C_BC = C_FOLD + M
C_GB = C_BC + PP
CW = C_GB + 2

USE_REMOTE_EXCHANGE = False   # XOR remote-DMA allreduce vs gpsimd collective

_built = None


def _build():
    import concourse.bass as bass
    import concourse.tile as tile
    from concourse import mybir
    import bass_rust

    f32 = mybir.dt.float32
    bf16 = mybir.dt.bfloat16
    Alu = mybir.AluOpType
    Act = mybir.ActivationFunctionType

    nc = bass.Bass("TRN2", target_bir_lowering=False, debug=False,
                   num_devices=N_CORES)

    xp_d = nc.dram_tensor("xp", [C, B * SPB], bf16, kind="ExternalInput").ap()
    attb_d = nc.dram_tensor("attb", [PP, SPB], bf16, kind="ExternalInput").ap()
    pn_d = nc.dram_tensor("pn", [PP, SPB], bf16, kind="ExternalInput").ap()
    pn0_d = nc.dram_tensor("pn0", [128, 1280], bf16, kind="ExternalInput").ap()
    hn0_d = nc.dram_tensor("hn0", [128, 1280], bf16, kind="ExternalInput").ap()
    wp_d = nc.dram_tensor("wpack", [128, 2 * MP], bf16, kind="ExternalInput").ap()
    cpack_d = nc.dram_tensor("cpack", [128, CW], f32, kind="ExternalInput").ap()

    out_d = nc.dram_tensor("out_main", [PP, SPB], bf16, kind="ExternalOutput").ap()
    out0_d = nc.dram_tensor("out0", [128, 1280], bf16, kind="ExternalOutput").ap()

    def pe_anchor(psum_tile, cp):
        # tiny matmul reading cp (seen by PE) writing one psum element:
        # absorbs the psum slot-release wait so real matmuls carry <=1 wait
        nc.tensor.matmul(psum_tile[0:1, 0:1], cp[0:1, 0:1], cp[0:1, 0:1],
                         start=True, stop=True, skip_group_check=True)

    with tile.TileContext(nc) as tc:
        with (
            tc.tile_pool(name="consts", bufs=1) as cpool,
            tc.tile_pool(name="attp", bufs=2) as attp,
            tc.tile_pool(name="xin", bufs=2) as xin,
            tc.tile_pool(name="ybuf", bufs=1) as ybuf,
            tc.tile_pool(name="sq", bufs=2) as sqp,
            tc.tile_pool(name="small", bufs=1) as sm,
            tc.tile_pool(name="pnl", bufs=2) as pnl,
            tc.tile_pool(name="p0l", bufs=1) as p0l,
            tc.tile_pool(name="obuf", bufs=2) as obuf,
            tc.tile_pool(name="zp", bufs=6, space="PSUM") as zp,
            tc.tile_pool(name="stp", bufs=1, space="PSUM") as stp,
            tc.tile_pool(name="dram", bufs=1, space="DRAM") as dr,
        ):
            exchg = sm.tile([PP, 16], f32, tag="exchg")
            # warmup collective: runs during phase 1 (no data deps) so the
            # ncfw/TOPSP path is hot when the real stats collective fires
            wcc_in = dr.tile([PP, 2], f32)
            wcc_out = dr.tile([PP * N_CORES, 2], f32)
            nc.gpsimd.collective_compute(
                "AllGather", mybir.AluOpType.bypass,
                replica_groups=[list(range(N_CORES))],
                ins=[wcc_in[:].opt()],
                outs=[wcc_out[:].opt()],
            )
            if USE_REMOTE_EXCHANGE:
                # ---- cross-core exchange setup (XOR allreduce) ----
                xsem = nc.alloc_semaphore("xsem")
                lsem = nc.alloc_semaphore("lsem")
                nc.gpsimd.sem_clear(xsem)
                nc.gpsimd.sem_clear(lsem)
                for j in range(1, 8):
                    rdests = [None] * 8
                    rdests[j] = (0, j)
                    nc.gpsimd.remote_dma_broadcast(
                        exchg[:, 2 * j:2 * j + 2], exchg[:, 0:2],
                        remote_sem=xsem, local_sem=lsem, rdests=rdests)

            # first-wave DMAs go out on the scalar engine's HWDGE queue: its
            # preamble finishes ~4us before sync's, so bytes flow earlier
            cp = cpool.tile([128, CW], f32)
            nc.scalar.dma_start(cp[:], cpack_d[:])
            wp = cpool.tile([128, 2 * MP], bf16)
            nc.scalar.dma_start(wp[:], wp_d[:])
            wt = [wp[:, 0:MP], wp[:, MP:2 * MP]]
            foldWt = cp[0:PP, C_FOLD:C_FOLD + M]
            bcWt = cp[0:M, C_BC:C_BC + PP]
            gam = cp[0:M, C_GB:C_GB + 1]      # 0.5*gamma (u|l)
            bet = cp[0:M, C_GB + 1:C_GB + 2]  # 0.5*beta

            y_full = ybuf.tile([PP, SPB], bf16)
            s1t = sm.tile([PP, (SPB // NB)], f32, tag="s1t")
            s2t = sm.tile([PP, (SPB // NB)], f32, tag="s2t")

            # ---- PE warm-up: ~3.5us of dummy matmuls trips the HAM into
            # the 2.4 GHz state before the first xp tile lands ----
            wz = zp.tile([PP, NB], f32, tag="z", name="warm_z")
            for _ in range(20):
                nc.tensor.matmul(wz[0:MP, 0:CW], cp[:, 0:MP], cp[:, 0:CW],
                                 start=True, stop=True, skip_group_check=True)

            # ---- phase 1: stream xp, matmul, y = z*a, accumulate sums ----
            for qs in range(QS):
                xq = {}
                for b in range(B):
                    for c in range(2):
                        t = xin.tile([128, XN], bf16, tag=f"x{b}{c}",
                                     name=f"x{b}{c}_{qs}")
                        lo = b * SPB + qs * XN
                        if qs == 0:
                            # split first super-iter loads: matmuls start on
                            # the first half while the second half streams
                            nc.scalar.dma_start(
                                t[:, 0:XN // 2],
                                xp_d[c * 128:(c + 1) * 128, lo:lo + XN // 2])
                            xdma = nc.scalar.dma_start(
                                t[:, XN // 2:XN],
                                xp_d[c * 128:(c + 1) * 128, lo + XN // 2:lo + XN])
                        else:
                            xdma = nc.sync.dma_start(
                                t[:], xp_d[c * 128:(c + 1) * 128, lo:lo + XN])
                        if qs == QS - 1 and b == B - 1 and c == 1:
                            last_xdma = xdma
                        xq[(b, c)] = t
                abt = attp.tile([PP, XN], bf16, tag="attb", name=f"attb_{qs}")
                if qs == 0:
                    nc.scalar.dma_start(abt[:, 0:XN // 2], attb_d[:, 0:XN // 2])
                    nc.scalar.dma_start(abt[:, XN // 2:XN],
                                        attb_d[:, XN // 2:XN])
                else:
                    nc.sync.dma_start(abt[:], attb_d[:, qs * XN:(qs + 1) * XN])

                for s in range(NW):              # 512-col z-windows
                    cs = slice(s * NB, (s + 1) * NB)
                    z = zp.tile([PP, NB], f32, tag="z", name=f"z_{qs}_{s}")
                    pe_anchor(z, cp)
                    # weight-outer order; the two batch images run on
                    # separate PE column halves concurrently
                    for c in range(2):
                        for b in range(B):
                            nc.tensor.matmul(z[b * MP:(b + 1) * MP, :],
                                             wt[c], xq[(b, c)][:, cs],
                                             start=(c == 0), stop=(c == 1),
                                             tile_position=(0, b * MP))
                    k = qs * NW + s
                    ys = slice(qs * XN + s * NB, qs * XN + (s + 1) * NB)
                    nc.vector.scalar_tensor_tensor(
                        out=y_full[:, ys], in0=z[:], scalar=1.0,
                        in1=abt[:, cs], op0=Alu.mult, op1=Alu.mult,
                        accum_out=s1t[:, k:k + 1])
                    sq = sqp.tile([PP, NB], bf16, tag="sq", name=f"sq_{qs}_{s}")
                    nc.scalar.activation(sq[:], y_full[:, ys], Act.Square,
                                         accum_out=s2t[:, k:k + 1])

            # ---- phase 2: reduce partials, XOR exchange, BN scale/bias ----
            from concourse.bass import _add_dep_helper
            prio = tc.high_priority()
            prio.__enter__()
            nc.vector.reduce_sum(exchg[:, 0:1], s1t[:], axis=mybir.AxisListType.X)
            nc.vector.reduce_sum(exchg[:, 1:2], s2t[:], axis=mybir.AxisListType.X)
            ar = sm.tile([PP, 2], f32, tag="ar")
            if USE_REMOTE_EXCHANGE:
                nc.gpsimd.trigger_dma(count=None)
                # Emitted with target 0 so the (single-core) Tile scheduler
                # sim doesn't deadlock; patched to >=14 post-scheduling below.
                xwait = nc.vector.wait_ge(xsem, 0)
                a8 = sm.tile([PP, 8], f32, tag="a8")
                add1 = nc.vector.tensor_add(a8[:], exchg[:, 0:8], exchg[:, 8:16])
                _add_dep_helper(add1.ins, xwait.ins, sync=True,
                                reason="gate local reduce on remote arrivals")
                a4 = sm.tile([PP, 4], f32, tag="a4")
                nc.vector.tensor_add(a4[:], a8[:, 0:4], a8[:, 4:8])
                nc.vector.tensor_add(ar[:], a4[:, 0:2], a4[:, 2:4])
            else:
                cc_in = dr.tile([PP, 2], f32)
                cc_out = dr.tile([PP * N_CORES, 2], f32)
                nc.sync.dma_start(cc_in[:], exchg[:, 0:2])
                nc.gpsimd.collective_compute(
                    "AllGather", mybir.AluOpType.bypass,
                    replica_groups=[list(range(N_CORES))],
                    ins=[cc_in[:].opt()],
                    outs=[cc_out[:].opt()],
                )
                nc.sync.dma_start(
                    exchg[:, 0:16],
                    cc_out[:].rearrange("(j p) e -> p j e", j=N_CORES))
                a8 = sm.tile([PP, 8], f32, tag="a8")
                nc.vector.tensor_add(a8[:], exchg[:, 0:8], exchg[:, 8:16])
                a4 = sm.tile([PP, 4], f32, tag="a4")
                nc.vector.tensor_add(a4[:], a8[:, 0:4], a8[:, 4:8])
                nc.vector.tensor_add(ar[:], a4[:, 0:2], a4[:, 2:4])

            folded = stp.tile([M, 2], f32, tag="folded")
            pe_anchor(folded, cp)
            nc.tensor.matmul(folded[:], foldWt, ar[:], start=True, stop=True)

            # foldW is pre-scaled by 1/NTOT on host: folded = (m, E[y^2])
            mE = sm.tile([M, 2], f32, tag="mE")
            nc.vector.tensor_copy(mE[:], folded[:])
            msq = sm.tile([M, 1], f32, tag="msq")
            nc.vector.tensor_mul(msq[:], mE[:, 0:1], mE[:, 0:1])
            vpe = sm.tile([M, 1], f32, tag="vpe")    # var + eps
            nc.vector.scalar_tensor_tensor(
                out=vpe[:], in0=mE[:, 1:2], scalar=EPS, in1=msq[:],
                op0=Alu.add, op1=Alu.subtract)
            sd = sm.tile([M, 1], f32, tag="sd")
            nc.scalar.activation(sd[:], vpe[:], Act.Sqrt)
            r = sm.tile([M, 1], f32, tag="r")
            nc.vector.reciprocal(r[:], sd[:])
            gh = sm.tile([M, 2], f32, tag="gh")      # (s', t') halved affine
            nc.vector.tensor_mul(gh[:, 0:1], r[:], gam)
            ms = sm.tile([M, 1], f32, tag="ms")
            nc.vector.tensor_mul(ms[:], mE[:, 0:1], gh[:, 0:1])
            nc.vector.tensor_sub(gh[:, 1:2], bet, ms[:])

            bc = stp.tile([PP, 2], f32, tag="bc")
            pe_anchor(bc, cp)
            nc.tensor.matmul(bc[:], bcWt, gh[:], start=True, stop=True)
            stb = sm.tile([PP, 2], f32, tag="stb")
            nc.scalar.copy(stb[:], bc[:])
            prio.__exit__(None, None, None)

            # ---- prefetch p_nodes during the exchange window ----
            pnt = {}
            for qs in range(QS):
                t = pnl.tile([PP, XN], bf16, tag="pn", name=f"pn_{qs}")
                pdma = nc.sync.dma_start(t[:], pn_d[:, qs * XN:(qs + 1) * XN])
                _add_dep_helper(pdma.ins, last_xdma.ins, sync=True,
                                reason="defer pn prefetch past xp stream")
                pnt[qs] = t

            # ---- background-node path (independent; overlaps exchange) ----
            pn0 = p0l.tile([128, 1280], bf16, tag="pn0")
            d1 = nc.sync.dma_start(pn0[:], pn0_d[:])
            hn0 = p0l.tile([128, 1280], bf16, tag="hn0")
            d2 = nc.sync.dma_start(hn0[:], hn0_d[:])
            _add_dep_helper(d1.ins, last_xdma.ins, sync=True,
                            reason="defer p0 loads past xp stream")
            _add_dep_helper(d2.ins, last_xdma.ins, sync=True,
                            reason="defer p0 loads past xp stream")
            # pn0/hn0 are pre-halved on host: out0 = pn0h + hn0h
            o0 = p0l.tile([128, 1280], bf16, tag="o0")
            nc.vector.tensor_add(o0[:], pn0[:], hn0[:])
            nc.sync.dma_start(out0_d[:], o0[:])

            # ---- phase 3: d = relu(s'*y + t') ; out = d + pn_half ----
            # pn is pre-halved on host. Work split across three engines:
            # ACT does relu-affine for 3 tiles, DVE (two 4x tensor_scalar
            # ops) for 5; the final adds go to GpSimd (5) and DVE (3).
            for ti in range(QS * (XN // NQ)):
                qs, s = divmod(ti, XN // NQ)
                ys = slice(qs * XN + s * NQ, qs * XN + (s + 1) * NQ)
                ps = slice(s * NQ, (s + 1) * NQ)
                d = obuf.tile([PP, NQ], bf16, tag="d", bufs=3,
                              name=f"d_{ti}")
                if ti % 8 in (0, 2, 3, 5, 7):  # 5 tiles on ACT
                    nc.scalar.activation(d[:], y_full[:, ys], Act.Relu,
                                         scale=stb[:, 0:1], bias=stb[:, 1:2])
                else:                         # 3 tiles on DVE (4x TS ops)
                    t1 = obuf.tile([PP, NQ], bf16, tag="t1", bufs=3,
                                   name=f"t1_{ti}")
                    nc.vector.tensor_scalar(
                        out=t1[:], in0=y_full[:, ys], scalar1=stb[:, 0:1],
                        scalar2=stb[:, 1:2], op0=Alu.mult, op1=Alu.add)
                    nc.vector.tensor_scalar_max(d[:], t1[:], 0.0)
                o = obuf.tile([PP, NQ], bf16, tag="o", bufs=3,
                              name=f"o_{ti}")
                if ti % 8 == 3:                # gpsimd TT is slow (~2.9us):
                    nc.gpsimd.tensor_add(o[:], pnt[qs][:, ps], d[:])
                else:                          # adds on DVE (2x TT, 594ns)
                    nc.vector.tensor_add(o[:], pnt[qs][:, ps], d[:])
                nc.sync.dma_start(out_d[:, ys], o[:])

    # patch the exchange wait to its real target (14 = 7 arrivals x 2):
    # it was emitted as wait_ge(xsem, 0) so the Tile scheduler sim passes
    if USE_REMOTE_EXCHANGE:
        import dataclasses
        n_patched = 0
        for fn in nc.m.functions:
            for bb in fn.blocks:
                for ins in bb.instructions:
                    si = ins.sync_info
                    if si is None:
                        continue
                    nw = []
                    changed = False
                    for w in si.on_wait:
                        if (getattr(w, 'ant_name', None) == 'xsem'
                                and getattr(w, 'wait_value', None) == 0):
                            nw.append(dataclasses.replace(w, wait_value=14))
                            changed = True
                            n_patched += 1
                        else:
                            nw.append(w)
                    if changed:
                        ins.sync_info = bass_rust.SyncInfo(
                            on_wait=nw, on_update=list(si.on_update))
        assert n_patched == 1, f"expected 1 xsem wait to patch, got {n_patched}"

    # hoist excess sync waits onto same-engine NOPs (walrus wait-slot limits)
    SI = bass_rust.SyncInfo
    k = 0
    for fn in nc.m.functions:
        for bb in fn.blocks:
            out = []
            for ins in bb.instructions:
                si = ins.sync_info
                if si is not None and len(si.on_wait) > 1:
                    waits = list(si.on_wait)
                    extra, keep = waits[:-1], waits[-1:]
                    for wti in extra:
                        nop = bass_rust.InstNoOp(name=f"Wsplit-{k}", ins=[], outs=[])
                        k += 1
                        nop.engine = ins.engine
                        nop.sync_info = SI(on_wait=[wti], on_update=[])
                        out.append(nop)
                    ins.sync_info = SI(on_wait=keep, on_update=list(si.on_update))
                out.append(ins)
            bb.instructions = out

    if USE_REMOTE_EXCHANGE:
        # gpsimd ucode library loads for the remote-DMA desc-gen ops
        from concourse.library_config import all_libraries, standard
        inst_type_to_lib_mask = {}
        for lib in all_libraries:
            for it in lib.instructions:
                inst_type_to_lib_mask[it] = (
                    inst_type_to_lib_mask.get(it, 0) | (1 << lib.index))
        bass_rust.insert_library_loads(
            nc, inst_type_to_lib_mask, len(all_libraries), standard.index)
    # populate .instr bytes for extended-ISA instructions (tensor_tensor_reduce)
    mybir.codegen_inst_isa_subclasses(nc)
    return nc


def _get_nc():
    global _built
    if _built is None:
        _built = _build()
    return _built


def _prep_core(i, p_nodes, h_nodes, xp, h_att, cpack, wpack):
    hs = i * HS
    bf = ml_dtypes.bfloat16
    xp_t = np.ascontiguousarray(
        xp[:, :, hs:hs + HS, :].transpose(1, 0, 2, 3)).reshape(
            C, B * SPB).astype(bf)
    attb = np.zeros((PP, SPB), bf)
    for b in range(B):
        attb[b * MP:b * MP + 40] = h_att[1, b, 0, hs:hs + HS, :].ravel()
        attb[b * MP + 40:b * MP + 60] = h_att[2, b, 0, hs:hs + HS, :].ravel()
    pn16 = 0.5 * p_nodes[1:7, :, :, hs:hs + HS, :]    # [6, B, 10, HS, W]
    pn16 = pn16.transpose(1, 0, 2, 3, 4).reshape(B, M, SPB)
    pn = np.zeros((PP, SPB), bf)
    pn[0:M] = pn16[0]
    pn[MP:MP + M] = pn16[1]
    pn0 = np.ascontiguousarray(
        0.5 * p_nodes[0, :, :, hs:hs + HS, :]).reshape(128, 1280).astype(bf)
    hn0 = np.ascontiguousarray(
        0.5 * h_nodes[0, :, :, hs:hs + HS, :]).reshape(128, 1280).astype(bf)
    return {"xp": xp_t, "attb": attb, "pn": pn,
            "pn0": pn0, "hn0": hn0, "cpack": cpack, "wpack": wpack}


def _make_consts(Wu, Wl, gamma_u, beta_u, gamma_l, beta_l):
    f32 = np.float32
    Wcat = np.concatenate([Wu, Wl], 0)                # [60, 256]
    lhsT = np.zeros((C, MP), f32)
    lhsT[:, 0:M] = Wcat.T
    wpack = np.zeros((128, 2 * MP), ml_dtypes.bfloat16)
    wpack[:, 0:MP] = lhsT[0:128]
    wpack[:, MP:2 * MP] = lhsT[128:256]
    cpack = np.zeros((128, CW), f32)
    foldW = np.zeros((PP, M), f32)
    foldW[0:M] = np.eye(M, dtype=f32) / NTOT
    foldW[MP:MP + M] = np.eye(M, dtype=f32) / NTOT
    cpack[0:PP, C_FOLD:C_FOLD + M] = foldW
    bcW = np.zeros((M, PP), f32)
    bcW[:, 0:M] = np.eye(M, dtype=f32)
    bcW[:, MP:MP + M] = np.eye(M, dtype=f32)
    cpack[0:M, C_BC:C_BC + PP] = bcW
    cpack[0:M, C_GB] = 0.5 * np.concatenate([gamma_u, gamma_l])
    cpack[0:M, C_GB + 1] = 0.5 * np.concatenate([beta_u, beta_l])
    return cpack, wpack


def _run(inputs, trace=False, trace_cores=None):
    from concourse import bass_utils
    p_nodes = np.asarray(inputs["p_nodes"], np.float32)
    h_nodes = np.asarray(inputs["h_nodes"], np.float32)
    xp = np.asarray(inputs["xp"], np.float32)
    h_att = np.asarray(inputs["h_att"], np.float32)
    cpack, wpack = _make_consts(np.asarray(inputs["Wu"], np.float32),
                                np.asarray(inputs["Wl"], np.float32),
                                np.asarray(inputs["gamma_u"], np.float32),
                                np.asarray(inputs["beta_u"], np.float32),
                                np.asarray(inputs["gamma_l"], np.float32),
                                np.asarray(inputs["beta_l"], np.float32))
    in_maps = [_prep_core(i, p_nodes, h_nodes, xp, h_att, cpack, wpack)
               for i in range(N_CORES)]
    nc = _get_nc()
    res = bass_utils.run_bass_kernel_spmd(
        nc, in_maps, core_ids=list(range(N_CORES)), trace=trace,
        trace_cores=trace_cores)

    p_new = np.empty((7, B, HID, H, W), np.float32)
    for i in range(N_CORES):
        hs = i * HS
        om = res.results[i]["out_main"].astype(np.float32)   # [128, SPB]
        o0 = res.results[i]["out0"].astype(np.float32)       # [128, 1280]
        p_new[0, :, :, hs:hs + HS, :] = o0.reshape(B, HID, HS, W)
        for b in range(B):
            blk = om[b * MP:b * MP + M].reshape(6, HID, HS, W)
            p_new[1:7, b, :, hs:hs + HS, :] = blk
    return p_new, res


def kernel(**inputs) -> np.ndarray:
    return _run(inputs, trace=False)[0]


# revision 34
# speedup vs baseline: 1.2174x; 1.2174x over previous
"""Trainium2 Bass kernel for nn_GNN_82781199663565 (gnn_message_passing).

Computation (see reference):
  du = relu(BN(einsum(h_att[1]*xp, Wu)))   # [B, 40, H, W]
  dl = relu(BN(einsum(h_att[2]*xp, Wl)))   # [B, 20, H, W]
  p_new[0]   = 0.5*(h_nodes[0] + p_nodes[0])
  p_new[1:5] = 0.5*(p_nodes[1:5] + du4)    # du reshaped to [4, B, 10, H, W]
  p_new[5:7] = 0.5*(p_nodes[5:7] + dl2)
(f_nodes, h_att[0], h_nodes[1:] are unused.)

Strategy: data-parallel over H (32 rows per core, 8 cores). Per core:
 - All bulk HBM traffic in bf16 (harness gate is rel_err < 2e-2): xp,
   attention, p_nodes, outputs. Stats/affine math stays f32.
 - One fused matmul z = Wcat.T @ xp per 512-col window; the two batch
   images run CONCURRENTLY on separate PE column halves via
   tile_position=(0,0)/(0,64) -> halves PE streaming time.
 - Attention applied after the conv (channel-independent) via one fused
   vector op that also accumulates the per-partition sum for BN stats.
 - Sync-BN via an XOR-pattern SBUF->SBUF remote-DMA exchange: each core
   broadcasts its [128,2] partial sums to peer (own_tpb ^ j) for
   j=1..7, receives 7 peer blocks, reduces locally. Replaces the
   gpsimd collective_compute AllReduce (~43us bubble -> a few us).
 - p_new = relu_affine(y) + 0.5*p_nodes in one fused vector op.
All host-side work is layout only (slice/transpose/pad/concat/cast).
"""
import sys
sys.path.insert(0, '/opt/trn_rl_repo')

import numpy as np
import ml_dtypes

N_CORES = 8
B, C, HID, H, W = 2, 256, 10, 256, 256
EPS = 1e-5
HS = H // N_CORES            # 32 H-rows per core
SPB = HS * W                 # spatial elems per batch image per core: 8192
M = 60                       # real output channels (40 u + 20 l)
MP = 64                      # padded to 64 -> groups tile partitions exactly
PP = 128
NQ = 1024                    # phase-3 columns per tile
NB = 512                     # matmul free-dim block (one PSUM bank, fp32)
NTOT = float(B * H * W)      # BN stat count: 131072
XN = 4096                    # xp super-tile columns (1 MiB bf16 DMAs)
QS = SPB // XN               # 2 super-iterations
NW = XN // NB                # 8 windows per super-iteration

# packed f32 constants column offsets: foldW, bcW, gamma, beta
C_FOLD = 0
C_BC = C_FOLD + M
C_GB = C_BC + PP
CW = C_GB + 2

USE_REMOTE_EXCHANGE = False   # XOR remote-DMA allreduce vs gpsimd collective

_built = None


def _build():
    import concourse.bass as bass
    import concourse.tile as tile
    from concourse import mybir
    import bass_rust

    f32 = mybir.dt.float32
    bf16 = mybir.dt.bfloat16
    Alu = mybir.AluOpType
    Act = mybir.ActivationFunctionType

    nc = bass.Bass("TRN2", target_bir_lowering=False, debug=False,
                   num_devices=N_CORES)

    xp_d = nc.dram_tensor("xp", [C, B * SPB], bf16, kind="ExternalInput").ap()
    attb_d = nc.dram_tensor("attb", [PP, SPB], bf16, kind="ExternalInput").ap()
    pn_d = nc.dram_tensor("pn", [PP, SPB], bf16, kind="ExternalInput").ap()
    pn0_d = nc.dram_tensor("pn0", [128, 1280], bf16, kind="ExternalInput").ap()
    hn0_d = nc.dram_tensor("hn0", [128, 1280], bf16, kind="ExternalInput").ap()
    wp_d = nc.dram_tensor("wpack", [128, 2 * MP], bf16, kind="ExternalInput").ap()
    cpack_d = nc.dram_tensor("cpack", [128, CW], f32, kind="ExternalInput").ap()

    out_d = nc.dram_tensor("out_main", [PP, SPB], bf16, kind="ExternalOutput").ap()
    out0_d = nc.dram_tensor("out0", [128, 1280], bf16, kind="ExternalOutput").ap()

    def pe_anchor(psum_tile, cp):
        # tiny matmul reading cp (seen by PE) writing one psum element:
        # absorbs the psum slot-release wait so real matmuls carry <=1 wait
        nc.tensor.matmul(psum_tile[0:1, 0:1], cp[0:1, 0:1], cp[0:1, 0:1],
                         start=True, stop=True, skip_group_check=True)

    with tile.TileContext(nc) as tc:
        with (
            tc.tile_pool(name="consts", bufs=1) as cpool,
            tc.tile_pool(name="attp", bufs=2) as attp,
            tc.tile_pool(name="xin", bufs=2) as xin,
            tc.tile_pool(name="ybuf", bufs=1) as ybuf,
            tc.tile_pool(name="sq", bufs=2) as sqp,
            tc.tile_pool(name="small", bufs=1) as sm,
            tc.tile_pool(name="pnl", bufs=2) as pnl,
            tc.tile_pool(name="p0l", bufs=1) as p0l,
            tc.tile_pool(name="obuf", bufs=2) as obuf,
            tc.tile_pool(name="zp", bufs=6, space="PSUM") as zp,
            tc.tile_pool(name="stp", bufs=1, space="PSUM") as stp,
            tc.tile_pool(name="dram", bufs=1, space="DRAM") as dr,
        ):
            exchg = sm.tile([PP, 16], f32, tag="exchg")
            # warmup collective: runs during phase 1 (no data deps) so the
            # ncfw/TOPSP path is hot when the real stats collective fires
            wcc_in = dr.tile([PP, 2], f32)
            wcc_out = dr.tile([PP * N_CORES, 2], f32)
            nc.gpsimd.collective_compute(
                "AllGather", mybir.AluOpType.bypass,
                replica_groups=[list(range(N_CORES))],
                ins=[wcc_in[:].opt()],
                outs=[wcc_out[:].opt()],
            )
            if USE_REMOTE_EXCHANGE:
                # ---- cross-core exchange setup (XOR allreduce) ----
                xsem = nc.alloc_semaphore("xsem")
                lsem = nc.alloc_semaphore("lsem")
                nc.gpsimd.sem_clear(xsem)
                nc.gpsimd.sem_clear(lsem)
                for j in range(1, 8):
                    rdests = [None] * 8
                    rdests[j] = (0, j)
                    nc.gpsimd.remote_dma_broadcast(
                        exchg[:, 2 * j:2 * j + 2], exchg[:, 0:2],
                        remote_sem=xsem, local_sem=lsem, rdests=rdests)

            # first-wave DMAs go out on the scalar engine's HWDGE queue: its
            # preamble finishes ~4us before sync's, so bytes flow earlier
            cp = cpool.tile([128, CW], f32)
            nc.scalar.dma_start(cp[:], cpack_d[:])
            wp = cpool.tile([128, 2 * MP], bf16)
            nc.scalar.dma_start(wp[:], wp_d[:])
            wt = [wp[:, 0:MP], wp[:, MP:2 * MP]]
            foldWt = cp[0:PP, C_FOLD:C_FOLD + M]
            bcWt = cp[0:M, C_BC:C_BC + PP]
            gam = cp[0:M, C_GB:C_GB + 1]      # 0.5*gamma (u|l)
            bet = cp[0:M, C_GB + 1:C_GB + 2]  # 0.5*beta

            y_full = ybuf.tile([PP, SPB], bf16)
            s1t = sm.tile([PP, (SPB // NB)], f32, tag="s1t")
            s2t = sm.tile([PP, (SPB // NB)], f32, tag="s2t")

            # ---- PE warm-up: ~3.5us of dummy matmuls trips the HAM into
            # the 2.4 GHz state before the first xp tile lands ----
            wz = zp.tile([PP, NB], f32, tag="z", name="warm_z")
            for _ in range(20):
                nc.tensor.matmul(wz[0:MP, 0:CW], cp[:, 0:MP], cp[:, 0:CW],
                                 start=True, stop=True, skip_group_check=True)

            # ---- phase 1: stream xp, matmul, y = z*a, accumulate sums ----
            for qs in range(QS):
                xq = {}
                for b in range(B):
                    for c in range(2):
                        t = xin.tile([128, XN], bf16, tag=f"x{b}{c}",
                                     name=f"x{b}{c}_{qs}")
                        lo = b * SPB + qs * XN
                        if qs == 0:
                            # split first super-iter loads: matmuls start on
                            # the first half while the second half streams
                            nc.scalar.dma_start(
                                t[:, 0:XN // 2],
                                xp_d[c * 128:(c + 1) * 128, lo:lo + XN // 2])
                            xdma = nc.scalar.dma_start(
                                t[:, XN // 2:XN],
                                xp_d[c * 128:(c + 1) * 128, lo + XN // 2:lo + XN])
                        else:
                            # split halves: windows 0-3 of this super-iter
                            # start while the second half still streams
                            nc.sync.dma_start(
                                t[:, 0:XN // 2],
                                xp_d[c * 128:(c + 1) * 128, lo:lo + XN // 2])
                            xdma = nc.sync.dma_start(
                                t[:, XN // 2:XN],
                                xp_d[c * 128:(c + 1) * 128, lo + XN // 2:lo + XN])
                        if qs == QS - 1 and b == B - 1 and c == 1:
                            last_xdma = xdma
                        xq[(b, c)] = t
                abt = attp.tile([PP, XN], bf16, tag="attb", name=f"attb_{qs}")
                if qs == 0:
                    nc.scalar.dma_start(abt[:, 0:XN // 2], attb_d[:, 0:XN // 2])
                    nc.scalar.dma_start(abt[:, XN // 2:XN],
                                        attb_d[:, XN // 2:XN])
                else:
                    nc.sync.dma_start(abt[:, 0:XN // 2],
                                      attb_d[:, qs * XN:qs * XN + XN // 2])
                    nc.sync.dma_start(abt[:, XN // 2:XN],
                                      attb_d[:, qs * XN + XN // 2:(qs + 1) * XN])

                for s in range(NW):              # 512-col z-windows
                    cs = slice(s * NB, (s + 1) * NB)
                    z = zp.tile([PP, NB], f32, tag="z", name=f"z_{qs}_{s}")
                    pe_anchor(z, cp)
                    # weight-outer order; the two batch images run on
                    # separate PE column halves concurrently
                    for c in range(2):
                        for b in range(B):
                            nc.tensor.matmul(z[b * MP:(b + 1) * MP, :],
                                             wt[c], xq[(b, c)][:, cs],
                                             start=(c == 0), stop=(c == 1),
                                             tile_position=(0, b * MP))
                    k = qs * NW + s
                    ys = slice(qs * XN + s * NB, qs * XN + (s + 1) * NB)
                    nc.vector.scalar_tensor_tensor(
                        out=y_full[:, ys], in0=z[:], scalar=1.0,
                        in1=abt[:, cs], op0=Alu.mult, op1=Alu.mult,
                        accum_out=s1t[:, k:k + 1])
                    sq = sqp.tile([PP, NB], bf16, tag="sq", name=f"sq_{qs}_{s}")
                    nc.scalar.activation(sq[:], y_full[:, ys], Act.Square,
                                         accum_out=s2t[:, k:k + 1])

            # ---- phase 2: reduce partials, XOR exchange, BN scale/bias ----
            from concourse.bass import _add_dep_helper
            prio = tc.high_priority()
            prio.__enter__()
            nc.vector.reduce_sum(exchg[:, 0:1], s1t[:], axis=mybir.AxisListType.X)
            nc.vector.reduce_sum(exchg[:, 1:2], s2t[:], axis=mybir.AxisListType.X)
            ar = sm.tile([PP, 2], f32, tag="ar")
            if USE_REMOTE_EXCHANGE:
                nc.gpsimd.trigger_dma(count=None)
                # Emitted with target 0 so the (single-core) Tile scheduler
                # sim doesn't deadlock; patched to >=14 post-scheduling below.
                xwait = nc.vector.wait_ge(xsem, 0)
                a8 = sm.tile([PP, 8], f32, tag="a8")
                add1 = nc.vector.tensor_add(a8[:], exchg[:, 0:8], exchg[:, 8:16])
                _add_dep_helper(add1.ins, xwait.ins, sync=True,
                                reason="gate local reduce on remote arrivals")
                a4 = sm.tile([PP, 4], f32, tag="a4")
                nc.vector.tensor_add(a4[:], a8[:, 0:4], a8[:, 4:8])
                nc.vector.tensor_add(ar[:], a4[:, 0:2], a4[:, 2:4])
            else:
                cc_in = dr.tile([PP, 2], f32)
                cc_out = dr.tile([PP * N_CORES, 2], f32)
                nc.sync.dma_start(cc_in[:], exchg[:, 0:2])
                nc.gpsimd.collective_compute(
                    "AllGather", mybir.AluOpType.bypass,
                    replica_groups=[list(range(N_CORES))],
                    ins=[cc_in[:].opt()],
                    outs=[cc_out[:].opt()],
                )
                nc.sync.dma_start(
                    exchg[:, 0:16],
                    cc_out[:].rearrange("(j p) e -> p j e", j=N_CORES))
                a8 = sm.tile([PP, 8], f32, tag="a8")
                nc.vector.tensor_add(a8[:], exchg[:, 0:8], exchg[:, 8:16])
                a4 = sm.tile([PP, 4], f32, tag="a4")
                nc.vector.tensor_add(a4[:], a8[:, 0:4], a8[:, 4:8])
                nc.vector.tensor_add(ar[:], a4[:, 0:2], a4[:, 2:4])

            folded = stp.tile([M, 2], f32, tag="folded")
            pe_anchor(folded, cp)
            nc.tensor.matmul(folded[:], foldWt, ar[:], start=True, stop=True)

            # foldW is pre-scaled by 1/NTOT on host: folded = (m, E[y^2])
            mE = sm.tile([M, 2], f32, tag="mE")
            nc.vector.tensor_copy(mE[:], folded[:])
            msq = sm.tile([M, 1], f32, tag="msq")
            nc.vector.tensor_mul(msq[:], mE[:, 0:1], mE[:, 0:1])
            vpe = sm.tile([M, 1], f32, tag="vpe")    # var + eps
            nc.vector.scalar_tensor_tensor(
                out=vpe[:], in0=mE[:, 1:2], scalar=EPS, in1=msq[:],
                op0=Alu.add, op1=Alu.subtract)
            sd = sm.tile([M, 1], f32, tag="sd")
            nc.scalar.activation(sd[:], vpe[:], Act.Sqrt)
            r = sm.tile([M, 1], f32, tag="r")
            nc.vector.reciprocal(r[:], sd[:])
            gh = sm.tile([M, 2], f32, tag="gh")      # (s', t') halved affine
            nc.vector.tensor_mul(gh[:, 0:1], r[:], gam)
            ms = sm.tile([M, 1], f32, tag="ms")
            nc.vector.tensor_mul(ms[:], mE[:, 0:1], gh[:, 0:1])
            nc.vector.tensor_sub(gh[:, 1:2], bet, ms[:])

            bc = stp.tile([PP, 2], f32, tag="bc")
            pe_anchor(bc, cp)
            nc.tensor.matmul(bc[:], bcWt, gh[:], start=True, stop=True)
            stb = sm.tile([PP, 2], f32, tag="stb")
            nc.scalar.copy(stb[:], bc[:])
            prio.__exit__(None, None, None)

            # ---- prefetch p_nodes during the exchange window ----
            pnt = {}
            for qs in range(QS):
                t = pnl.tile([PP, XN], bf16, tag="pn", name=f"pn_{qs}")
                pdma = nc.sync.dma_start(t[:], pn_d[:, qs * XN:(qs + 1) * XN])
                _add_dep_helper(pdma.ins, last_xdma.ins, sync=True,
                                reason="defer pn prefetch past xp stream")
                pnt[qs] = t

            # ---- background-node path (independent; overlaps exchange) ----
            pn0 = p0l.tile([128, 1280], bf16, tag="pn0")
            d1 = nc.sync.dma_start(pn0[:], pn0_d[:])
            hn0 = p0l.tile([128, 1280], bf16, tag="hn0")
            d2 = nc.sync.dma_start(hn0[:], hn0_d[:])
            _add_dep_helper(d1.ins, last_xdma.ins, sync=True,
                            reason="defer p0 loads past xp stream")
            _add_dep_helper(d2.ins, last_xdma.ins, sync=True,
                            reason="defer p0 loads past xp stream")
            # pn0/hn0 are pre-halved on host: out0 = pn0h + hn0h
            o0 = p0l.tile([128, 1280], bf16, tag="o0")
            nc.vector.tensor_add(o0[:], pn0[:], hn0[:])
            nc.sync.dma_start(out0_d[:], o0[:])

            # ---- phase 3: d = relu(s'*y + t') ; out = d + pn_half ----
            # pn is pre-halved on host. Work split across three engines:
            # ACT does relu-affine for 3 tiles, DVE (two 4x tensor_scalar
            # ops) for 5; the final adds go to GpSimd (5) and DVE (3).
            for ti in range(QS * (XN // NQ)):
                qs, s = divmod(ti, XN // NQ)
                ys = slice(qs * XN + s * NQ, qs * XN + (s + 1) * NQ)
                ps = slice(s * NQ, (s + 1) * NQ)
                d = obuf.tile([PP, NQ], bf16, tag="d", bufs=3,
                              name=f"d_{ti}")
                if ti % 8 in (0, 2, 3, 5, 7):  # 5 tiles on ACT
                    nc.scalar.activation(d[:], y_full[:, ys], Act.Relu,
                                         scale=stb[:, 0:1], bias=stb[:, 1:2])
                else:                         # 3 tiles on DVE (4x TS ops)
                    t1 = obuf.tile([PP, NQ], bf16, tag="t1", bufs=3,
                                   name=f"t1_{ti}")
                    nc.vector.tensor_scalar(
                        out=t1[:], in0=y_full[:, ys], scalar1=stb[:, 0:1],
                        scalar2=stb[:, 1:2], op0=Alu.mult, op1=Alu.add)
                    nc.vector.tensor_scalar_max(d[:], t1[:], 0.0)
                o = obuf.tile([PP, NQ], bf16, tag="o", bufs=3,
                              name=f"o_{ti}")
                if ti % 8 == 3:                # gpsimd TT is slow (~2.9us):
                    nc.gpsimd.tensor_add(o[:], pnt[qs][:, ps], d[:])
                else:                          # adds on DVE (2x TT, 594ns)
                    nc.vector.tensor_add(o[:], pnt[qs][:, ps], d[:])
                nc.sync.dma_start(out_d[:, ys], o[:])

    # patch the exchange wait to its real target (14 = 7 arrivals x 2):
    # it was emitted as wait_ge(xsem, 0) so the Tile scheduler sim passes
    if USE_REMOTE_EXCHANGE:
        import dataclasses
        n_patched = 0
        for fn in nc.m.functions:
            for bb in fn.blocks:
                for ins in bb.instructions:
                    si = ins.sync_info
                    if si is None:
                        continue
                    nw = []
                    changed = False
                    for w in si.on_wait:
                        if (getattr(w, 'ant_name', None) == 'xsem'
                                and getattr(w, 'wait_value', None) == 0):
                            nw.append(dataclasses.replace(w, wait_value=14))
                            changed = True
                            n_patched += 1
                        else:
                            nw.append(w)
                    if changed:
                        ins.sync_info = bass_rust.SyncInfo(
                            on_wait=nw, on_update=list(si.on_update))
        assert n_patched == 1, f"expected 1 xsem wait to patch, got {n_patched}"

    # hoist excess sync waits onto same-engine NOPs (walrus wait-slot limits)
    SI = bass_rust.SyncInfo
    k = 0
    for fn in nc.m.functions:
        for bb in fn.blocks:
            out = []
            for ins in bb.instructions:
                si = ins.sync_info
                if si is not None and len(si.on_wait) > 1:
                    waits = list(si.on_wait)
                    extra, keep = waits[:-1], waits[-1:]
                    for wti in extra:
                        nop = bass_rust.InstNoOp(name=f"Wsplit-{k}", ins=[], outs=[])
                        k += 1
                        nop.engine = ins.engine
                        nop.sync_info = SI(on_wait=[wti], on_update=[])
                        out.append(nop)
                    ins.sync_info = SI(on_wait=keep, on_update=list(si.on_update))
                out.append(ins)
            bb.instructions = out

    if USE_REMOTE_EXCHANGE:
        # gpsimd ucode library loads for the remote-DMA desc-gen ops
        from concourse.library_config import all_libraries, standard
        inst_type_to_lib_mask = {}
        for lib in all_libraries:
            for it in lib.instructions:
                inst_type_to_lib_mask[it] = (
                    inst_type_to_lib_mask.get(it, 0) | (1 << lib.index))
        bass_rust.insert_library_loads(
            nc, inst_type_to_lib_mask, len(all_libraries), standard.index)
    # populate .instr bytes for extended-ISA instructions (tensor_tensor_reduce)
    mybir.codegen_inst_isa_subclasses(nc)
    return nc


def _get_nc():
    global _built
    if _built is None:
        _built = _build()
    return _built


def _prep_core(i, p_nodes, h_nodes, xp, h_att, cpack, wpack):
    hs = i * HS
    bf = ml_dtypes.bfloat16
    xp_t = np.ascontiguousarray(
        xp[:, :, hs:hs + HS, :].transpose(1, 0, 2, 3)).reshape(
            C, B * SPB).astype(bf)
    attb = np.zeros((PP, SPB), bf)
    for b in range(B):
        attb[b * MP:b * MP + 40] = h_att[1, b, 0, hs:hs + HS, :].ravel()
        attb[b * MP + 40:b * MP + 60] = h_att[2, b, 0, hs:hs + HS, :].ravel()
    pn16 = 0.5 * p_nodes[1:7, :, :, hs:hs + HS, :]    # [6, B, 10, HS, W]
    pn16 = pn16.transpose(1, 0, 2, 3, 4).reshape(B, M, SPB)
    pn = np.zeros((PP, SPB), bf)
    pn[0:M] = pn16[0]
    pn[MP:MP + M] = pn16[1]
    pn0 = np.ascontiguousarray(
        0.5 * p_nodes[0, :, :, hs:hs + HS, :]).reshape(128, 1280).astype(bf)
    hn0 = np.ascontiguousarray(
        0.5 * h_nodes[0, :, :, hs:hs + HS, :]).reshape(128, 1280).astype(bf)
    return {"xp": xp_t, "attb": attb, "pn": pn,
            "pn0": pn0, "hn0": hn0, "cpack": cpack, "wpack": wpack}


def _make_consts(Wu, Wl, gamma_u, beta_u, gamma_l, beta_l):
    f32 = np.float32
    Wcat = np.concatenate([Wu, Wl], 0)                # [60, 256]
    lhsT = np.zeros((C, MP), f32)
    lhsT[:, 0:M] = Wcat.T
    wpack = np.zeros((128, 2 * MP), ml_dtypes.bfloat16)
    wpack[:, 0:MP] = lhsT[0:128]
    wpack[:, MP:2 * MP] = lhsT[128:256]
    cpack = np.zeros((128, CW), f32)
    foldW = np.zeros((PP, M), f32)
    foldW[0:M] = np.eye(M, dtype=f32) / NTOT
    foldW[MP:MP + M] = np.eye(M, dtype=f32) / NTOT
    cpack[0:PP, C_FOLD:C_FOLD + M] = foldW
    bcW = np.zeros((M, PP), f32)
    bcW[:, 0:M] = np.eye(M, dtype=f32)
    bcW[:, MP:MP + M] = np.eye(M, dtype=f32)
    cpack[0:M, C_BC:C_BC + PP] = bcW
    cpack[0:M, C_GB] = 0.5 * np.concatenate([gamma_u, gamma_l])
    cpack[0:M, C_GB + 1] = 0.5 * np.concatenate([beta_u, beta_l])
    return cpack, wpack


def _run(inputs, trace=False, trace_cores=None):
    from concourse import bass_utils
    p_nodes = np.asarray(inputs["p_nodes"], np.float32)
    h_nodes = np.asarray(inputs["h_nodes"], np.float32)
    xp = np.asarray(inputs["xp"], np.float32)
    h_att = np.asarray(inputs["h_att"], np.float32)
    cpack, wpack = _make_consts(np.asarray(inputs["Wu"], np.float32),
                                np.asarray(inputs["Wl"], np.float32),
                                np.asarray(inputs["gamma_u"], np.float32),
                                np.asarray(inputs["beta_u"], np.float32),
                                np.asarray(inputs["gamma_l"], np.float32),
                                np.asarray(inputs["beta_l"], np.float32))
    in_maps = [_prep_core(i, p_nodes, h_nodes, xp, h_att, cpack, wpack)
               for i in range(N_CORES)]
    nc = _get_nc()
    res = bass_utils.run_bass_kernel_spmd(
        nc, in_maps, core_ids=list(range(N_CORES)), trace=trace,
        trace_cores=trace_cores)

    p_new = np.empty((7, B, HID, H, W), np.float32)
    for i in range(N_CORES):
        hs = i * HS
        om = res.results[i]["out_main"].astype(np.float32)   # [128, SPB]
        o0 = res.results[i]["out0"].astype(np.float32)       # [128, 1280]
        p_new[0, :, :, hs:hs + HS, :] = o0.reshape(B, HID, HS, W)
        for b in range(B):
            blk = om[b * MP:b * MP + M].reshape(6, HID, HS, W)
            p_new[1:7, b, :, hs:hs + HS, :] = blk
    return p_new, res


def kernel(**inputs) -> np.ndarray:
    return _run(inputs, trace=False)[0]
